# revision 15
# baseline (speedup 1.0000x reference)
"""AdditiveOscillator Trainium2 kernel (8-core data parallel, batch per core).

Pipeline per core (batch b):
  amp   = softplus(W@z + b) * (n*f0 < SR/2)              [PE + ACT + DVE, tiny]
  amp_up= lerp-upsample(amp)   as a rank-4-per-chunk matmul on PE
  c     = cumsum(f0_up/SR)     blocked scan: DVE scan + PE tri-matmul,
                               int part dropped (mod-1 invariant), kept accurate
  y     = n * c_pos            PE broadcast-matmul
  u     = mod(y,1) - 0.5       DVE (exact residue; sin(2pi u) = -sin(2pi frac))
  s     = Sin(2pi * u)         ACT
  G     = amp-window^T @ s     PE  (8 rows per chunk, stacked 4 chunks/bank)
  harm  = -sum_m C*G           DVE product + PE block-reduce
Outputs: harmonic [1,64000], amp_up [64,64000] per core.

Packed layout: partition p<64 -> (q=0, h=p), p>=64 -> (q=1, h=p-64);
q=0 covers j in [0,32256), q=1 covers j in [32256,64512) (tail >=64000 junk).
"""

import sys

for _p in ("/opt/trn_rl_repo", "/root/.axon_site", "/root/.axon_site/_ro/pypackages"):
    if _p not in sys.path:
        sys.path.append(_p)

import numpy as np

SR = 24000
HOP = 256
T = 250
H = 64
CDIM = 256
B = 8
TOUT = T * HOP            # 64000
CH = 512                  # chunk width
NCH = 63                  # chunks per packed half
WHALF = NCH * CH          # 32256
NBLK = 128                # cumsum blocks (125 used)
TWO23 = np.float32(1.5 * 2.0 ** 23)  # round magic, valid for |y| < 2^21
TWO_PI_DOWN = np.nextafter(np.float32(2 * np.pi), np.float32(0.0))

f32 = np.float32


def _static_tables():
    pos = np.linspace(0.0, T - 1.0, TOUT)
    i0 = np.floor(pos).astype(np.int64)
    i1 = np.minimum(i0 + 1, T - 1)
    w = (pos - i0).astype(f32)
    omw = (f32(1.0) - w).astype(f32)

    # --- cumsum-block f0_up coefficient tables: x[p,c], j = 512p + c ---
    KP = np.zeros(NBLK, np.int64)
    CC = np.zeros((4, NBLK, CH), f32)
    FIDX = np.zeros((NBLK, 4), np.int64)
    for p in range(125):
        j0 = p * CH
        KP[p] = i0[j0]
        for m in range(4):
            FIDX[p, m] = min(KP[p] + m, T - 1)
        jj = np.arange(j0, j0 + CH)
        for m in range(4):
            fr = KP[p] + m
            CC[m, p] = omw[jj] * (i0[jj] == fr) + w[jj] * (i1[jj] == fr)
    FIDX[125:] = 0

    # --- per-chunk frame windows for the packed halves ---
    F0 = np.zeros(NCH, np.int64)
    F1 = np.zeros(NCH, np.int64)
    # Cq[q, cc, m, r]
    Cq = np.zeros((2, NCH, 4, CH), f32)
    for cc in range(NCH):
        F0[cc] = i0[CH * cc]
        jj = np.arange(CH * cc, CH * cc + CH)
        for m in range(4):
            fr = F0[cc] + m
            Cq[0, cc, m] = omw[jj] * (i0[jj] == fr) + w[jj] * (i1[jj] == fr)
        jstart = WHALF + CH * cc
        if jstart < TOUT:
            F1[cc] = i0[jstart]
            jj = np.arange(jstart, min(jstart + CH, TOUT))
            r = jj - jstart
            for m in range(4):
                fr = F1[cc] + m
                Cq[1, cc, m, r] = omw[jj] * (i0[jj] == fr) + w[jj] * (i1[jj] == fr)
        else:
            F1[cc] = 246  # dummy, coefficients stay zero
    assert F0.max() + 3 <= 127, F0.max()
    assert F1.min() >= 125 and F1.max() + 3 <= 252, (F1.min(), F1.max())

    # --- MM1 rhs table: call[64, 4096] ---
    call = np.zeros((64, 8 * CH), f32)
    for cc in range(NCH):
        s = 4 * (cc % 8)
        cb = CH * (cc // 8)
        for m in range(4):
            call[s + m, cb:cb + CH] = Cq[0, cc, m]
            call[32 + s + m, cb:cb + CH] = Cq[1, cc, m]

    # --- product-pass table: cstack[128, 8192] ---
    cstack = np.zeros((128, 16 * CH), f32)
    for cc in range(NCH):
        S, g = cc // 4, cc % 4
        for q in range(2):
            for m in range(4):
                cstack[32 * g + 4 * q + m, CH * S:CH * S + CH] = Cq[q, cc, m]

    # --- MM2 stationary: lhst2[128, 2048], replicated over 32-row groups ---
    nvals = np.arange(1, H + 1, dtype=f32)
    lhst2 = np.zeros((128, 16 * 128), f32)
    for t in range(16):
        for p in range(128):
            k = 2 * t + (p // 64)
            for u in range(4):
                lhst2[32 * u + k, 128 * t + p] = nvals[p % 64]

    # --- MM4 stationary: lhst4[128, 8] (negated block-ones) ---
    lhst4 = np.zeros((128, 8), f32)
    for g in range(4):
        for q in range(2):
            lhst4[32 * g + 4 * q:32 * g + 4 * q + 4, 2 * g + q] = f32(1.0)

    tri = np.tril(np.ones((128, 128), f32), -1).T.copy()  # TRI[k,p]=1 iff k<p
    iden = np.eye(128, dtype=f32)
    nvec = nvals.reshape(H, 1).copy()

    import ml_dtypes
    bf16 = ml_dtypes.bfloat16
    call = call.astype(bf16)
    lhst4 = lhst4.astype(bf16)
    idneg = (-np.eye(128, dtype=f32)).astype(bf16)

    return dict(CC=CC, FIDX=FIDX, F0=F0, F1=F1, call=call, cstack=cstack,
                lhst2=lhst2, lhst4=lhst4, tri=tri, iden=iden, nvec=nvec,
                idneg=idneg)


_TAB = None
_NC = None


def _tables():
    global _TAB
    if _TAB is None:
        _TAB = _static_tables()
    return _TAB


def _build_module():
    import concourse.bacc as bacc
    import concourse.mybir as mybir
    from concourse.tile import TileContext

    tab = _tables()
    F0, F1 = tab["F0"], tab["F1"]
    dt = mybir.dt
    Alu = mybir.AluOpType
    Act = mybir.ActivationFunctionType

    nc = bacc.Bacc("TRN2", target_bir_lowering=False)
    z_d = nc.declare_dram_parameter("z", [CDIM, T], dt.float32, isOutput=False)
    wt_d = nc.declare_dram_parameter("wt", [CDIM, H], dt.float32, isOutput=False)
    bias_d = nc.declare_dram_parameter("bias", [H, 1], dt.float32, isOutput=False)
    f0b_d = nc.declare_dram_parameter("f0b", [H, T], dt.float32, isOutput=False)
    f0g_d = nc.declare_dram_parameter("f0g", [NBLK, 4], dt.float32, isOutput=False)
    cc_d = nc.declare_dram_parameter("cc", [NBLK, 4 * CH], dt.float32, isOutput=False)
    call_d = nc.declare_dram_parameter("call", [64, 8 * CH], dt.bfloat16, isOutput=False)
    cstack_d = nc.declare_dram_parameter("cstack", [128, 16 * CH], dt.float32, isOutput=False)
    lhst2_d = nc.declare_dram_parameter("lhst2", [128, 2048], dt.float32, isOutput=False)
    lhst4_d = nc.declare_dram_parameter("lhst4", [128, 8], dt.bfloat16, isOutput=False)
    idneg_d = nc.declare_dram_parameter("idneg", [128, 128], dt.bfloat16, isOutput=False)
    tri_d = nc.declare_dram_parameter("tri", [128, 128], dt.float32, isOutput=False)
    iden_d = nc.declare_dram_parameter("iden", [128, 128], dt.float32, isOutput=False)
    nvec_d = nc.declare_dram_parameter("nvec", [H, 1], dt.float32, isOutput=False)
    out_a = nc.declare_dram_parameter("out_a", [H, TOUT], dt.float32, isOutput=True)
    out_h = nc.declare_dram_parameter("out_h", [1, TOUT], dt.float32, isOutput=True)

    with TileContext(nc) as tc:
        with tc.tile_pool(name="persist", bufs=1) as pp:
            # ---- constant + input loads ----
            za = pp.tile([128, T], dt.float32, name="za")
            zb = pp.tile([128, T], dt.float32, name="zb")
            wta = pp.tile([128, H], dt.float32, name="wta")
            wtb = pp.tile([128, H], dt.float32, name="wtb")
            bias_t = pp.tile([H, 1], dt.float32, name="bias_t")
            f0b_t = pp.tile([H, T], dt.float32, name="f0b_t")
            f0g_t = pp.tile([NBLK, 4], dt.float32, name="f0g_t")
            cc_t = pp.tile([NBLK, 4 * CH], dt.float32, name="cc_t")
            call_t = pp.tile([64, 8 * CH], dt.bfloat16, name="call_t")
            cstack_t = pp.tile([128, 16 * CH], dt.float32, name="cstack_t")
            lhst2_t = pp.tile([128, 2048], dt.float32, name="lhst2_t")
            lhst4_t = pp.tile([128, 8], dt.bfloat16, name="lhst4_t")
            idneg_t = pp.tile([128, 128], dt.bfloat16, name="idneg_t")
            tri_t = pp.tile([128, 128], dt.float32, name="tri_t")
            iden_t = pp.tile([128, 128], dt.float32, name="iden_t")
            nvec_t = pp.tile([H, 1], dt.float32, name="nvec_t")

            nc.sync.dma_start(out=za, in_=z_d[0:128, :])
            nc.sync.dma_start(out=zb, in_=z_d[128:256, :])
            nc.sync.dma_start(out=wta, in_=wt_d[0:128, :])
            nc.sync.dma_start(out=wtb, in_=wt_d[128:256, :])
            nc.sync.dma_start(out=bias_t, in_=bias_d[:, :])
            nc.sync.dma_start(out=f0b_t, in_=f0b_d[:, :])
            nc.sync.dma_start(out=f0g_t, in_=f0g_d[:, :])
            nc.sync.dma_start(out=cc_t, in_=cc_d[:, :])
            nc.sync.dma_start(out=call_t, in_=call_d[:, :])
            nc.sync.dma_start(out=cstack_t, in_=cstack_d[:, :])
            nc.sync.dma_start(out=lhst2_t, in_=lhst2_d[:, :])
            nc.sync.dma_start(out=lhst4_t, in_=lhst4_d[:, :])
            nc.sync.dma_start(out=idneg_t, in_=idneg_d[:, :])
            nc.sync.dma_start(out=tri_t, in_=tri_d[:, :])
            nc.sync.dma_start(out=iden_t, in_=iden_d[:, :])
            nc.sync.dma_start(out=nvec_t, in_=nvec_d[:, :])

            # ---- working persistent tiles ----
            amp_sb = pp.tile([H, 256], dt.float32, name="amp_sb")
            msk = pp.tile([H, T], dt.float32, name="msk")
            amp2 = pp.tile([H, 256], dt.float32, name="amp2")
            ampT_a = pp.tile([128, H], dt.bfloat16, name="ampT_a")
            ampT_b = pp.tile([128, H], dt.bfloat16, name="ampT_b")
            amp2b = pp.tile([H, 256], dt.bfloat16, name="amp2b")
            x_a = pp.tile([NBLK, CH], dt.float32, name="x_a")
            x_b = pp.tile([NBLK, CH], dt.float32, name="x_b")
            intra = pp.tile([NBLK, CH], dt.float32, name="intra")
            bt2 = pp.tile([NBLK, 2], dt.float32, name="bt2")
            offs_sb = pp.tile([NBLK, 2], dt.float32, name="offs_sb")
            c_raw = pp.tile([NBLK, CH], dt.float32, name="c_raw")
            c_int = pp.tile([NBLK, CH], dt.float32, name="c_int")
            c_pos = pp.tile([NBLK, CH], dt.float32, name="c_pos")
            wide32 = pp.tile([128, CH], dt.float32, name="wide32")
            lhst1_all = pp.tile([64, 128 * NCH], dt.bfloat16, name="lhst1_all")
            lhst3_all = pp.tile([128, 8 * NCH], dt.bfloat16, name="lhst3_all")

            nc.gpsimd.memset(lhst1_all, 0.0)
            nc.gpsimd.memset(lhst3_all, 0.0)
            nc.gpsimd.memset(wide32, 0.0)
            nc.vector.memset(amp2, 0.0)

            with tc.tile_pool(name="pro_ps", bufs=1, space="PSUM") as prps:
                # ---- amp = softplus(W@z + b) * mask ----
                amp_ps = prps.tile([H, T], dt.float32, name="amp_ps")
                nc.tensor.matmul(amp_ps, wta, za, start=True, stop=False)
                nc.tensor.matmul(amp_ps, wtb, zb, start=False, stop=True)
                # softplus(x) = relu(x) + ln(1 + exp(-|x|)), x = W@z + bias
                sp_abs = pp.tile([H, T], dt.float32, name="sp_abs")
                sp_rel = pp.tile([H, T], dt.float32, name="sp_rel")
                nc.scalar.activation(sp_abs, amp_ps, Act.Abs,
                                     bias=bias_t[:, 0:1], scale=1.0)
                nc.scalar.activation(sp_rel, amp_ps, Act.Relu,
                                     bias=bias_t[:, 0:1], scale=1.0)
                nc.scalar.activation(sp_abs, sp_abs, Act.Exp,
                                     bias=0.0, scale=-1.0)
                nc.scalar.activation(sp_abs, sp_abs, Act.Ln,
                                     bias=1.0, scale=1.0)
                nc.vector.tensor_tensor(amp_sb[:, 0:T], sp_rel, sp_abs, Alu.add)
                nc.vector.tensor_scalar(msk, f0b_t, nvec_t[:, 0:1], 12000.0,
                                        Alu.mult, Alu.is_lt)
                nc.vector.tensor_tensor(amp2[:, 0:T], amp_sb[:, 0:T], msk, Alu.mult)

                # ---- transposes of amp for MM1 stationary windows ----
                tpa = prps.tile([128, H], dt.float32, name="tpa")
                tpb = prps.tile([128, H], dt.float32, name="tpb")
                nc.tensor.transpose(tpa, amp2[:, 0:128], iden_t[0:64, 0:64])
                nc.tensor.transpose(tpb, amp2[:, 125:253], iden_t[0:64, 0:64])
                nc.vector.tensor_copy(ampT_a, tpa)
                nc.vector.tensor_copy(ampT_b, tpb)
                nc.vector.tensor_copy(amp2b, amp2)

                # ---- x = f0_up / SR  in [128, 512] block layout ----
                nc.vector.tensor_scalar(x_a, cc_t[:, 0:CH], f0g_t[:, 0:1], None,
                                        Alu.mult)
                nc.vector.scalar_tensor_tensor(x_b, cc_t[:, CH:2 * CH],
                                               f0g_t[:, 1:2], x_a,
                                               Alu.mult, Alu.add)
                nc.vector.scalar_tensor_tensor(x_a, cc_t[:, 2 * CH:3 * CH],
                                               f0g_t[:, 2:3], x_b,
                                               Alu.mult, Alu.add)
                nc.vector.scalar_tensor_tensor(x_b, cc_t[:, 3 * CH:4 * CH],
                                               f0g_t[:, 3:4], x_a,
                                               Alu.mult, Alu.add)
                nc.vector.tensor_scalar(x_a, x_b, float(np.float32(1.0) / np.float32(SR)), None,
                                        Alu.mult)

                # ---- blocked cumsum with int/frac-split offsets ----
                nc.vector.tensor_tensor_scan(intra, x_a, x_a, 0.0,
                                             Alu.add, Alu.bypass)
                nc.vector.tensor_scalar(bt2[:, 0:1], intra[:, CH - 1:CH],
                                        float(TWO23), -float(TWO23),
                                        Alu.add, Alu.add)
                nc.vector.scalar_tensor_tensor(bt2[:, 1:2], intra[:, CH - 1:CH],
                                               1.0, bt2[:, 0:1],
                                               Alu.mult, Alu.subtract)
                offs_ps = prps.tile([128, 2], dt.float32, name="offs_ps")
                nc.tensor.matmul(offs_ps, tri_t, bt2, start=True, stop=True)
                nc.vector.tensor_copy(offs_sb, offs_ps)
                nc.vector.tensor_scalar(c_raw, intra, offs_sb[:, 1:2], None,
                                        Alu.add)
                nc.vector.tensor_scalar(c_int, c_raw, float(TWO23),
                                        -float(TWO23), Alu.add, Alu.add)
                nc.vector.scalar_tensor_tensor(c_pos, c_raw, 1.0, c_int,
                                               Alu.mult, Alu.subtract)

            # ---- repack c_pos blocks into wide32 rows ----
            for b4 in range(4):
                for t in range(16):
                    for q in range(2):
                        cc = b4 + 4 * t
                        if cc >= NCH:
                            continue
                        src = NCH * q + cc
                        nc.sync.dma_start(
                            out=wide32[32 * b4 + 2 * t + q:32 * b4 + 2 * t + q + 1, :],
                            in_=c_pos[src:src + 1, :])

            # ---- patch per-chunk stationaries ----
            for cc in range(NCH):
                s = 4 * (cc % 8)
                nc.sync.dma_start(
                    out=lhst1_all[s:s + 4, 128 * cc:128 * cc + 64],
                    in_=ampT_a[F0[cc]:F0[cc] + 4, :])
                nc.sync.dma_start(
                    out=lhst1_all[32 + s:32 + s + 4, 128 * cc + 64:128 * cc + 128],
                    in_=ampT_b[F1[cc] - 125:F1[cc] - 121, :])
                nc.sync.dma_start(
                    out=lhst3_all[0:64, 8 * cc:8 * cc + 4],
                    in_=amp2b[0:64, F0[cc]:F0[cc] + 4])
                nc.sync.dma_start(
                    out=lhst3_all[64:128, 8 * cc + 4:8 * cc + 8],
                    in_=amp2b[0:64, F1[cc]:F1[cc] + 4])

            # ---- main loop ----
            with tc.tile_pool(name="ampu_ps", bufs=2, space="PSUM") as pA, \
                 tc.tile_pool(name="y_ps", bufs=2, space="PSUM") as pY, \
                 tc.tile_pool(name="g_ps", bufs=1, space="PSUM") as pG, \
                 tc.tile_pool(name="h_ps", bufs=1, space="PSUM") as pH, \
                 tc.tile_pool(name="k_sb", bufs=2) as pK, \
                 tc.tile_pool(name="s_sb", bufs=2) as pS, \
                 tc.tile_pool(name="au_sb", bufs=3) as pAs, \
                 tc.tile_pool(name="pr_sb", bufs=2) as pP, \
                 tc.tile_pool(name="hm_sb", bufs=2) as pHs:

                y_t = None
                s_t = None

                for cc in range(NCH):
                    b4, t16 = cc % 4, cc // 4

                    # MM1: amp_up chunk
                    au_ps = pA.tile([128, CH], dt.float32, name="au_ps")
                    nc.tensor.matmul(
                        au_ps, lhst1_all[:, 128 * cc:128 * (cc + 1)],
                        call_t[:, CH * (cc // 8):CH * (cc // 8 + 1)],
                        start=True, stop=True)

                    # MM2: y = n * c_red  (pairs of chunks share a 1024 psum tile)
                    if cc % 2 == 0:
                        y_t = pY.tile([128, 2 * CH], dt.float32, name="y_t")
                    nc.tensor.matmul(
                        y_t[:, CH * (cc % 2):CH * (cc % 2 + 1)],
                        lhst2_t[32 * b4:32 * b4 + 32, 128 * t16:128 * (t16 + 1)],
                        wide32[32 * b4:32 * b4 + 32, :],
                        start=True, stop=False, tile_position=(32 * b4, 0),
                        skip_group_check=True)

                    # frac: k = round(y) on DVE (bf16 ints exact), then
                    # y -= k via -I matmul, so the psum tile becomes u.
                    # sin reads psum directly; group of 4 chunks -> one s tile.
                    if cc % 4 == 0:
                        s_t = pS.tile([128, 4 * CH], dt.bfloat16, name="s_t")
                    if cc % 2 == 1 or cc == NCH - 1:
                        pair = (cc // 2) % 2
                        wcols = CH * (cc % 2 + 1)
                        k_t = pK.tile([128, 2 * CH], dt.bfloat16, name="k_t")
                        nc.vector.tensor_scalar(
                            k_t[:, 0:wcols], y_t[:, 0:wcols], float(TWO23),
                            -float(TWO23), Alu.add, Alu.add)
                        for i in range(cc % 2 + 1):
                            nc.tensor.matmul(
                                y_t[:, CH * i:CH * (i + 1)], idneg_t,
                                k_t[:, CH * i:CH * (i + 1)],
                                start=False, stop=True, skip_group_check=True)
                        nc.scalar.activation(
                            s_t[:, 2 * CH * pair:2 * CH * pair + wcols],
                            y_t[:, 0:wcols], Act.Sin,
                            bias=0.0, scale=float(TWO_PI_DOWN))

                    # amp_up copy out (alternate engines) + DMA
                    au_sb = pAs.tile([128, CH], dt.float32, name="au_sb")
                    if cc % 2 == 0:
                        nc.scalar.copy(au_sb, au_ps)
                    else:
                        nc.vector.tensor_copy(au_sb, au_ps)
                    nc.sync.dma_start(out=out_a[0:64, CH * cc:CH * (cc + 1)],
                                      in_=au_sb[0:64, :])
                    if cc < NCH - 1:
                        nc.sync.dma_start(
                            out=out_a[0:64, WHALF + CH * cc:WHALF + CH * (cc + 1)],
                            in_=au_sb[64:128, :])

                    # group tail: G + product + harmonic for chunks S*4..cc
                    if cc % 4 == 3 or cc == NCH - 1:
                        S = cc // 4
                        ccs = list(range(4 * S, cc + 1))
                        g_t = pG.tile([128, CH], dt.float32, name="g_t")
                        if S <= 1:
                            nc.vector.memset(g_t, 0.0)
                        for c2 in ccs:
                            g = c2 % 4
                            nc.tensor.matmul(
                                g_t[32 * g:32 * g + 8, :],
                                lhst3_all[:, 8 * c2:8 * (c2 + 1)],
                                s_t[:, CH * g:CH * (g + 1)],
                                start=True, stop=True, tile_position=(0, 32 * g))
                        prod = pP.tile([128, CH], dt.bfloat16, name="prod")
                        nc.vector.tensor_tensor(prod, g_t,
                                                cstack_t[:, CH * S:CH * (S + 1)],
                                                Alu.mult)
                        hm_ps = pH.tile([8, CH], dt.float32, name="hm_ps")
                        nc.tensor.matmul(hm_ps, lhst4_t, prod,
                                         start=True, stop=True)
                        hm_sb = pHs.tile([8, CH], dt.float32, name="hm_sb")
                        nc.scalar.copy(hm_sb, hm_ps)
                        for c2 in ccs:
                            g = c2 % 4
                            nc.sync.dma_start(
                                out=out_h[0:1, CH * c2:CH * (c2 + 1)],
                                in_=hm_sb[2 * g:2 * g + 1, :])
                            if c2 < NCH - 1:
                                nc.sync.dma_start(
                                    out=out_h[0:1,
                                              WHALF + CH * c2:WHALF + CH * (c2 + 1)],
                                    in_=hm_sb[2 * g + 1:2 * g + 2, :])

    nc.finalize()
    return nc


def _get_module():
    global _NC
    if _NC is None:
        _NC = _build_module()
    return _NC


def _per_core_inputs(z, f0, W, b, core):
    tab = _tables()
    zb_ = np.ascontiguousarray(z[core].astype(f32))                  # [256, 250]
    f0c = f0[core, 0].astype(f32)                                    # [250]
    in_map = {
        "z": zb_,
        "wt": np.ascontiguousarray(W.astype(f32).T),                 # [256, 64]
        "bias": b.astype(f32).reshape(H, 1).copy(),
        "f0b": np.ascontiguousarray(np.broadcast_to(f0c, (H, T))),
        "f0g": np.ascontiguousarray(f0c[tab["FIDX"]]),               # [128, 4]
        "cc": np.ascontiguousarray(
            tab["CC"].transpose(1, 0, 2).reshape(NBLK, 4 * CH)),
        "call": tab["call"],
        "cstack": tab["cstack"],
        "lhst2": tab["lhst2"],
        "lhst4": tab["lhst4"],
        "idneg": tab["idneg"],
        "tri": tab["tri"],
        "iden": tab["iden"],
        "nvec": tab["nvec"],
    }
    return in_map


def run_on_hw(z, f0, W, b, trace=False):
    from concourse.bass_utils import run_bass_kernel_spmd

    nc = _get_module()
    core_ids = list(range(B))
    in_maps = [_per_core_inputs(z, f0, W, b, c) for c in core_ids]
    res = run_bass_kernel_spmd(nc, in_maps, core_ids, trace=trace)
    harm = np.zeros((B, 1, TOUT), f32)
    ampu = np.zeros((B, H, TOUT), f32)
    for c in core_ids:
        harm[c, 0] = res.results[c]["out_h"][0]
        ampu[c] = res.results[c]["out_a"]
    return (harm, ampu), res


def kernel(z, f0, W, b):
    out, _ = run_on_hw(np.asarray(z), np.asarray(f0), np.asarray(W),
                       np.asarray(b), trace=False)
    return out
